# revision 15
# baseline (speedup 1.0000x reference)
"""CgpHmmCell forward-algorithm kernel for Trainium2 (8 NeuronCores).

Sharding: data-parallel over batch (32 seqs -> 4 per core), params replicated.

State space S=612 folded as s = 17*r + c (r<36, c<17); 36*17==612 exactly, so
the mod-612 wrap of the banded transition (offsets {0,1,5,17}) is a clean
circular shift of r, implemented as a tiny persistent matmul on the PE.

Phase 1: one-hot inputs -> E in folded layout (PE transpose + 17 fold-matmuls
         against Bm column-slices) -> DRAM scratch.
Phase 2: 4095-step scan, two independent 2-seq chains interleaved so the
         per-step PE->ACT->DVE->PE->ACT->GPS chain pipelines across engines.
         Deferred-exact normalization every 16 steps (+ forced before the
         final step so alphaT matches the reference bit-for-bit-ish).
"""

import numpy as np

S = 612
COLS = 100
AB = 4
NG = COLS // AB
B_TOT = 32
T_FULL = 4096
NCORES = 8
SEQ = 4
RF, CF = 36, 17
OFFS = (0, 1, 5, 17)
# o-block order inside the W / mprod tiles (chosen so the two DVE mults use
# uniform-stride views of EXT: pair A = {1,0} (ext offsets 16,17, step +1),
# pair B = {17,5} (ext offsets 0,12, step +12))
O_ORDER = (1, 0, 17, 5)
NORM_K = 16
CHUNK = 128
W2 = 34  # per-seq ext width (c~ in [-17, 17))


# ----------------------------------------------------------------------------
# host-side parameter prep
# ----------------------------------------------------------------------------

def _seg_softmax_np(v, seg, n):
    v = np.asarray(v, np.float32)
    seg = np.asarray(seg, np.int64)
    m = np.full(n, -np.inf, np.float32)
    np.maximum.at(m, seg, v)
    e = np.exp(v - m[seg])
    s = np.zeros(n, np.float32)
    np.add.at(s, seg, e)
    return e / s[seg]


def _prep_params(init_kernel, transition_kernel, emission_kernel,
                 A_rows, A_cols, B_rows, B_cols, I_states):
    A_rows = np.asarray(A_rows, np.int64)
    A_cols = np.asarray(A_cols, np.int64)
    B_rows = np.asarray(B_rows, np.int64)
    B_cols = np.asarray(B_cols, np.int64)

    vA = _seg_softmax_np(transition_kernel, A_rows, S)
    off = (A_cols - A_rows) % S
    w = {}
    for o in OFFS:
        wo = np.zeros(S, np.float32)
        sel = off == o
        wo[A_rows[sel]] = vA[sel]
        w[o] = wo

    segB = B_rows * NG + B_cols // AB
    vB = _seg_softmax_np(emission_kernel, segB, S * NG)
    Bm = np.zeros((S, COLS), np.float32)
    Bm[B_rows, B_cols] = vB
    Bm = Bm.T.copy()  # [COLS, S]

    ik = np.asarray(init_kernel, np.float32)
    e = np.exp(ik - ik.max())
    Ivec = np.zeros(S, np.float32)
    Ivec[np.asarray(I_states, np.int64)] = (e / e.sum()).astype(np.float32)

    idx = 17 * np.arange(RF)[:, None] + np.arange(CF)[None, :]  # [36,17]

    # W tile [RF, (o4, q2, c17)] with o in O_ORDER, q replicated x2 (per chain)
    wf = np.zeros((RF, 4, 2, CF), np.float32)
    for oi, o in enumerate(O_ORDER):
        wf[:, oi, :, :] = w[o][(idx - o) % S][:, None, :]
    wf = wf.reshape(RF, 4 * 2 * CF)

    # fold-matmul stationaries bmw[:, c*RF + r] = Bm[:, 17 r + c]
    bmw = np.zeros((COLS, CF * RF), np.float32)
    for c in range(CF):
        bmw[:, c * RF:(c + 1) * RF] = Bm[:, idx[:, c]]

    ifold = Ivec[idx]  # [36, 17]
    ifold_rep = np.repeat(ifold[:, None, :], SEQ, axis=1).reshape(RF, SEQ * CF)

    sh = np.zeros((RF, RF), np.float32)
    sh[np.arange(RF), (np.arange(RF) + 1) % RF] = 1.0  # lhsT[k,m]=1 iff m=k+1
    eye36 = np.eye(RF, dtype=np.float32)
    ones36 = np.ones((RF, 1), np.float32)

    return dict(wf=wf, bmw=bmw, ifold=ifold_rep, sh=sh, eye36=eye36,
                ones36=ones36)


# ----------------------------------------------------------------------------
# device kernel emission
# ----------------------------------------------------------------------------

def _view(t, off, dims):
    from concourse.bass import AP
    a = t.copy()
    return AP(a.tensor, a.offset + off, [a.ap[0]] + [list(d) for d in dims])


def build_nc(T):
    import concourse.bacc as bacc
    import concourse.mybir as mybir
    from concourse import bass
    from concourse.tile import TileContext
    from concourse.masks import make_identity

    fp32 = mybir.dt.float32
    Alu = mybir.AluOpType
    ActF = mybir.ActivationFunctionType
    nchunks = T // CHUNK
    assert T % CHUNK == 0 and nchunks >= 1

    nc = bacc.Bacc()
    x_in = nc.declare_dram_parameter("x", [SEQ, T, COLS], fp32, isOutput=False)
    wf_in = nc.declare_dram_parameter("wf", [RF, 4 * 2 * CF], fp32, isOutput=False)
    bmw_in = nc.declare_dram_parameter("bmw", [COLS, CF * RF], fp32, isOutput=False)
    ifold_in = nc.declare_dram_parameter("ifold", [RF, SEQ * CF], fp32, isOutput=False)
    sh_in = nc.declare_dram_parameter("sh", [RF, RF], fp32, isOutput=False)
    eye36_in = nc.declare_dram_parameter("eye36", [RF, RF], fp32, isOutput=False)
    ones36_in = nc.declare_dram_parameter("ones36", [RF, 1], fp32, isOutput=False)

    alpha_out = nc.declare_dram_parameter("alpha", [RF, SEQ * CF], fp32, isOutput=True)
    ll_out = nc.declare_dram_parameter("ll", [1, SEQ], fp32, isOutput=True)

    efold = nc.dram_tensor("efold", [nchunks, RF, SEQ, CF * CHUNK], fp32)

    from contextlib import ExitStack
    with TileContext(nc) as tc, ExitStack() as _stack:
        with (
            tc.tile_pool(name="consts", bufs=1) as cpool,
            tc.tile_pool(name="p2sbuf", bufs=1) as p2,
        ):
            wf = cpool.tile([RF, 4 * 2 * CF], fp32)
            bmw = cpool.tile([COLS, CF * RF], fp32)
            ifold = cpool.tile([RF, SEQ * CF], fp32)
            sh = cpool.tile([RF, RF], fp32)
            eye36 = cpool.tile([RF, RF], fp32)
            ones36 = cpool.tile([RF, 1], fp32)
            ones_row = cpool.tile([1, RF], fp32)
            eye128 = cpool.tile([128, 128], fp32)
            nc.sync.dma_start(wf[:], wf_in[:])
            nc.sync.dma_start(bmw[:], bmw_in[:])
            nc.sync.dma_start(ifold[:], ifold_in[:])
            nc.sync.dma_start(sh[:], sh_in[:])
            nc.sync.dma_start(eye36[:], eye36_in[:])
            nc.sync.dma_start(ones36[:], ones36_in[:])
            make_identity(nc, eye128[:])
            nc.vector.memset(ones_row[:], 1.0)

            # ---------------- phase 1 ----------------
            with (
                tc.tile_pool(name="p1sbuf", bufs=3) as p1s,
                tc.tile_pool(name="p1psum", bufs=1, space="PSUM") as p1p,
            ):
                def p1_body(k):
                    for q in range(SEQ):
                        xin = p1s.tile([CHUNK, COLS], fp32, tag="xin")
                        nc.sync.dma_start(
                            xin[:], x_in[q, bass.ts(k, CHUNK), :])
                        trp = p1p.tile([COLS, CHUNK], fp32, tag="trp")
                        nc.tensor.transpose(trp[:], xin[:], eye128[:])
                        ot = p1s.tile([COLS, CHUNK], fp32, tag="ot")
                        nc.scalar.copy(ot[:], trp[:])
                        fps = p1p.tile([RF, CF * CHUNK], fp32, tag="fps")
                        for c in range(CF):
                            nc.tensor.matmul(
                                out=fps[:, c * CHUNK:(c + 1) * CHUNK],
                                lhsT=bmw[:, c * RF:(c + 1) * RF],
                                rhs=ot[:],
                                start=True, stop=True)
                        fsb = p1s.tile([RF, CF * CHUNK], fp32, tag="fsb")
                        nc.scalar.copy(fsb[:], fps[:])
                        dst = efold[k, :, q, :]
                        nc.sync.dma_start(dst, fsb[:])

                if nchunks > 1:
                    with tc.For_i(0, nchunks) as k:
                        p1_body(k)
                else:
                    p1_body(0)

            # ---------------- phase 2 ----------------
            p2p = _stack.enter_context(
                tc.tile_pool(name="p2psum", bufs=1, space="PSUM"))
            ext = [p2.tile([RF, 2 * W2], fp32, tag=f"ext{c}", name=f"ext{c}") for c in range(2)]
            mpr = [p2.tile([RF, 4 * 2 * CF], fp32, tag=f"mp{c}", name=f"mp{c}") for c in range(2)]
            ebuf = p2.tile([RF, CHUNK * SEQ * CF], fp32, tag="ebuf")
            ll_acc = p2.tile([1, SEQ], fp32, tag="ll")
            s_red = [p2.tile([1, 2], fp32, tag=f"sred{c}", name=f"sred{c}") for c in range(2)]
            s_rec = [p2.tile([1, 2], fp32, tag=f"srec{c}", name=f"srec{c}") for c in range(2)]

            logs = [p2.tile([1, 2], fp32, tag=f"logs{c}", name=f"logs{c}") for c in range(2)]

            shp = [p2p.tile([RF, W2], fp32, tag=f"shp{c}", name=f"shp{c}") for c in range(2)]
            rp = [p2p.tile([RF, W2], fp32, tag=f"rp{c}", name=f"rp{c}") for c in range(2)]
            sup = [p2p.tile([1, W2], fp32, tag=f"sup{c}", name=f"sup{c}") for c in range(2)]
            s_bc = [p2p.tile([RF, 2], fp32, tag=f"sbc{c}", name=f"sbc{c}") for c in range(2)]

            nc.vector.memset(ll_acc[:], 0.0)

            def ext_u(ch):  # [RF, (q2, c17)] strided view
                return _view(ext[ch][:], CF, [[W2, 2], [1, CF]])

            def ext_lo(ch):
                return _view(ext[ch][:], 0, [[W2, 2], [1, CF]])

            def e_slice(ch, t):  # [RF, (q2, c17)] strided view into ebuf
                # ebuf layout: [r, (q:SEQ, c:CF, t:CHUNK)]
                base = ch * 2 * CF * CHUNK + t
                return _view(ebuf[:], base, [[CF * CHUNK, 2], [CHUNK, CF]])

            def load_echunk(k):
                src = efold[k, :, :, :].rearrange("r q w -> r (q w)")
                nc.sync.dma_start(ebuf[:], src)

            def emit_norm(ch):
                # sums over (r, c) per q of current u
                nc.tensor.matmul(out=sup[ch][:], lhsT=ones36[:],
                                 rhs=ext_u(ch), start=True, stop=True)
                nc.vector.tensor_reduce(
                    out=s_red[ch][:],
                    in_=_view(sup[ch][:], 0, [[CF, 2], [1, CF]]),
                    axis=mybir.AxisListType.X, op=Alu.add)
                nc.scalar.activation(
                    out=logs[ch][:], in_=s_red[ch][:], func=ActF.Ln)
                nc.vector.tensor_tensor(
                    out=ll_acc[:, ch * 2:(ch + 1) * 2],
                    in0=ll_acc[:, ch * 2:(ch + 1) * 2],
                    in1=logs[ch][:], op=Alu.add)
                nc.vector.reciprocal(out=s_rec[ch][:], in_=s_red[ch][:])
                nc.tensor.matmul(out=s_bc[ch][:], lhsT=ones_row[:],
                                 rhs=s_rec[ch][:], start=True, stop=True)
                nc.vector.tensor_tensor(
                    out=ext_u(ch), in0=ext_u(ch),
                    in1=_view(s_bc[ch][:], 0, [[1, 2], [0, CF]]),
                    op=Alu.mult)

            def emit_step(ch, t_in_chunk):
                # 1. PE ring shift: shp[r] = u[r-1]
                nc.tensor.matmul(out=shp[ch][:], lhsT=sh[:], rhs=ext_u(ch),
                                 start=True, stop=True)
                # 2. ACT: ext-lo <- shp
                nc.scalar.copy(ext_lo(ch),
                               _view(shp[ch][:], 0, [[CF, 2], [1, CF]]))
                # 3/4. DVE mults (pairs {1,0} and {17,5})
                nc.vector.tensor_tensor(
                    out=mpr[ch][:, 0:2 * 2 * CF].rearrange(
                        "r (o q c) -> r o q c", o=2, q=2),
                    in0=wf[:, 0:2 * 2 * CF].rearrange(
                        "r (o q c) -> r o q c", o=2, q=2),
                    in1=_view(ext[ch][:], 16, [[1, 2], [W2, 2], [1, CF]]),
                    op=Alu.mult)
                nc.vector.tensor_tensor(
                    out=mpr[ch][:, 2 * 2 * CF:].rearrange(
                        "r (o q c) -> r o q c", o=2, q=2),
                    in0=wf[:, 2 * 2 * CF:].rearrange(
                        "r (o q c) -> r o q c", o=2, q=2),
                    in1=_view(ext[ch][:], 0, [[12, 2], [W2, 2], [1, CF]]),
                    op=Alu.mult)
                # 5. PE sum of the 4 o-blocks
                for b in range(4):
                    nc.tensor.matmul(
                        out=rp[ch][:], lhsT=eye36[:],
                        rhs=mpr[ch][:, b * W2:(b + 1) * W2],
                        start=(b == 0), stop=(b == 3))
                # 6. DVE: u <- rp * e_t  (PSUM read)
                nc.vector.tensor_tensor(
                    out=ext_u(ch),
                    in0=_view(rp[ch][:], 0, [[CF, 2], [1, CF]]),
                    in1=e_slice(ch, t_in_chunk), op=Alu.mult)

            def emit_chunk(norm_ts, step_ts, init=False, final=False):
                """norm_ts/step_ts are t_in_chunk lists; t global = offset+t."""
                for ch in range(2):
                    if init:
                        nc.vector.tensor_tensor(
                            out=ext_u(ch),
                            in0=_view(ifold[:], ch * 2 * CF, [[CF, 2], [1, CF]]),
                            in1=e_slice(ch, 0), op=Alu.mult)
                for t in step_ts:
                    for ch in range(2):
                        if t in norm_ts:
                            emit_norm(ch)
                        emit_step(ch, t)

            # chunk 0
            load_echunk(0)
            norm0 = set(range(NORM_K, CHUNK, NORM_K))
            if nchunks == 1:
                norm0 = norm0 | {CHUNK - 1}
            emit_chunk(norm0, list(range(1, CHUNK)), init=True)
            # middle chunks
            norm_mid = set(range(0, CHUNK, NORM_K))
            if nchunks > 2:
                with tc.For_i(1, nchunks - 1) as k:
                    load_echunk(k)
                    emit_chunk(norm_mid, list(range(CHUNK)))
            # final chunk
            if nchunks > 1:
                load_echunk(nchunks - 1)
                emit_chunk(norm_mid | {CHUNK - 1}, list(range(CHUNK)),
                           final=True)

            # epilogue: ll_final = ll_acc + log(sum(u))
            for ch in range(2):
                nc.tensor.matmul(out=sup[ch][:], lhsT=ones36[:],
                                 rhs=ext_u(ch), start=True, stop=True)
                nc.vector.tensor_reduce(
                    out=s_red[ch][:],
                    in_=_view(sup[ch][:], 0, [[CF, 2], [1, CF]]),
                    axis=mybir.AxisListType.X, op=Alu.add)
                nc.scalar.activation(
                    out=logs[ch][:], in_=s_red[ch][:], func=ActF.Ln)
                nc.vector.tensor_tensor(
                    out=ll_acc[:, ch * 2:(ch + 1) * 2],
                    in0=ll_acc[:, ch * 2:(ch + 1) * 2],
                    in1=logs[ch][:], op=Alu.add)
                nc.sync.dma_start(
                    alpha_out[:].rearrange("r (q c) -> r q c", q=SEQ)
                    [:, ch * 2:(ch + 1) * 2, :],
                    ext_u(ch))
            nc.sync.dma_start(ll_out[:], ll_acc[:])

    nc.compile()
    return nc


# ----------------------------------------------------------------------------
# public entry point
# ----------------------------------------------------------------------------

_NC_CACHE = {}


def _get_nc(T):
    if T not in _NC_CACHE:
        _NC_CACHE[T] = build_nc(T)
    return _NC_CACHE[T]


def kernel(inputs, init_kernel, transition_kernel, emission_kernel,
           A_rows, A_cols, B_rows, B_cols, I_states):
    from concourse.bass_utils import run_bass_kernel_spmd

    inputs = np.asarray(inputs, np.float32)
    B, T, _ = inputs.shape
    assert B == B_TOT and T == T_FULL, (B, T)

    prm = _prep_params(init_kernel, transition_kernel, emission_kernel,
                       A_rows, A_cols, B_rows, B_cols, I_states)
    nc = _get_nc(T)

    in_maps = []
    for core in range(NCORES):
        xs = inputs[core * SEQ:(core + 1) * SEQ]  # [4, T, 100]
        in_maps.append(dict(
            x=np.ascontiguousarray(xs),
            wf=prm["wf"], bmw=prm["bmw"], ifold=prm["ifold"], sh=prm["sh"],
            eye36=prm["eye36"], ones36=prm["ones36"],
        ))
    res = run_bass_kernel_spmd(nc, in_maps, list(range(NCORES)))

    alphaT = np.zeros((B_TOT, S), np.float32)
    loglik = np.zeros((B_TOT,), np.float32)
    for core in range(NCORES):
        a = res.results[core]["alpha"].reshape(RF, SEQ, CF)  # [36, 4, 17]
        alphaT[core * SEQ:(core + 1) * SEQ] = (
            a.transpose(1, 0, 2).reshape(SEQ, S))
        loglik[core * SEQ:(core + 1) * SEQ] = res.results[core]["ll"][0]
    return alphaT, loglik
